# revision 1
# baseline (speedup 1.0000x reference)
import numpy as np
import jax
import jax.numpy as jnp
from jax.scipy.special import logsumexp

# nn_LstmCrf problem constants (hardcoded; kernel.py must be self-contained)
VOCAB, EMB, HID, S, B = 50000, 300, 512, 200, 64
N_TAGS = 64
N_LABELS = N_TAGS + 2
START, STOP = N_LABELS - 2, N_LABELS - 1
MAX_NORM = 6.0
N_CORES = 8
B_SH = B // N_CORES  # 8 sequences per core


def _shard_fn(data, lengths, labels, emb_table, W_ih, W_hh, b, W_fc, b_fc, transitions):
    # data:[b,S] int32, lengths:[b], labels:[b,S]
    # Embedding with max_norm renorm, applied only to gathered rows:
    # (table*scale)[data] == table[data]*scale[data]
    rows = emb_table[data]  # [b,S,E]
    norms = jnp.sqrt(jnp.sum(rows * rows, axis=2, keepdims=True))
    scale = jnp.minimum(1.0, MAX_NORM / jnp.maximum(norms, 1e-7))
    emb = rows * scale

    x_proj = jnp.einsum('bse,ge->bsg', emb, W_ih) + b  # [b,S,4H]

    def lstm_step(carry, xt):
        h, c = carry
        gates = xt + h @ W_hh.T
        i, f, g, o = jnp.split(gates, 4, axis=-1)
        c = jax.nn.sigmoid(f) * c + jax.nn.sigmoid(i) * jnp.tanh(g)
        h = jax.nn.sigmoid(o) * jnp.tanh(c)
        return (h, c), h

    h0 = jnp.zeros((emb.shape[0], HID), emb.dtype)
    _, hs = jax.lax.scan(lstm_step, (h0, h0), jnp.swapaxes(x_proj, 0, 1))
    h = jnp.swapaxes(hs, 0, 1)  # [b,S,H]

    feats = h @ W_fc.T + b_fc  # [b,S,N_LABELS]

    # CRF forward (log partition)
    bsz = feats.shape[0]
    alpha0 = jnp.full((bsz, N_LABELS), -10000.0).at[:, START].set(0.0)

    def crf_step(alpha, inp):
        logit, t = inp
        alpha_nxt = logsumexp(transitions[None, :, :] + alpha[:, None, :], axis=2) + logit
        alpha = jnp.where((t < lengths)[:, None], alpha_nxt, alpha)
        return alpha, None

    alpha, _ = jax.lax.scan(
        crf_step, alpha0,
        (jnp.swapaxes(feats, 0, 1), jnp.arange(S, dtype=lengths.dtype)))
    norm = logsumexp(alpha + transitions[STOP][None, :], axis=1)  # [b]

    # transition score
    ext = jnp.concatenate([
        jnp.full((bsz, 1), START, labels.dtype), labels,
        jnp.full((bsz, 1), STOP, labels.dtype)], axis=1)  # [b,S+2]
    pos = jnp.arange(S + 2, dtype=lengths.dtype)
    ext = jnp.where(pos[None, :] < (lengths + 1)[:, None], ext, STOP)
    trn = transitions[ext[:, 1:], ext[:, :-1]]  # [b,S+1]
    mask = (jnp.arange(S + 1, dtype=lengths.dtype)[None, :] < (lengths + 1)[:, None]).astype(trn.dtype)
    t_score = (trn * mask).sum(1)

    # features score
    scr = jnp.take_along_axis(feats, labels[:, :, None], axis=2)[:, :, 0]
    fmask = (jnp.arange(S, dtype=lengths.dtype)[None, :] < lengths[:, None]).astype(scr.dtype)
    f_score = (scr * fmask).sum(1)

    return norm - (t_score + f_score)


_pmapped = jax.pmap(
    _shard_fn,
    in_axes=(0, 0, 0, None, None, None, None, None, None, None),
    devices=jax.devices()[:N_CORES],
)


def kernel(data, lengths, labels, emb_table, W_ih, W_hh, b, W_fc, b_fc, transitions):
    # Full unsharded inputs -> shard batch across 8 cores -> full output [B]
    data = np.asarray(data).astype(np.int32).reshape(N_CORES, B_SH, S)
    lengths_sh = np.asarray(lengths).astype(np.int32).reshape(N_CORES, B_SH)
    labels = np.asarray(labels).astype(np.int32).reshape(N_CORES, B_SH, S)
    emb_table = np.asarray(emb_table, dtype=np.float32)
    W_ih = np.asarray(W_ih, dtype=np.float32)
    W_hh = np.asarray(W_hh, dtype=np.float32)
    b = np.asarray(b, dtype=np.float32)
    W_fc = np.asarray(W_fc, dtype=np.float32)
    b_fc = np.asarray(b_fc, dtype=np.float32)
    transitions = np.asarray(transitions, dtype=np.float32)

    out = _pmapped(data, lengths_sh, labels, emb_table, W_ih, W_hh, b,
                   W_fc, b_fc, transitions)
    return np.asarray(out).reshape(B).astype(np.float32)
